# revision 11
# baseline (speedup 1.0000x reference)
"""Bipolar dense layer on 8 Trainium2 NeuronCores.

Computes out = relu(x @ sign(w) + b) for x:[8192,4096] f32, w:[4096,4096] f32,
b:[4096] f32.

Strategy: data-parallel over the batch dim — each of the 8 cores gets a
[1024, 4096] shard of x (host passes it pre-transposed to [4096, 1024] so the
contraction dim lands on SBUF partitions), plus a full copy of w and b.

Per core (computing the TRANSPOSED output outT = [units, batch_shard]):
  - x shard is loaded once, cast fp32->bf16, and kept resident in SBUF (8 MB);
    its [128, 512] k-tiles are the matmul's moving operand.
  - w is streamed in [128, 512] fp32 tiles; sign() runs on the scalar engine
    (ACT) with a bf16 output — sign values {-1, 0, +1} are exact in bf16.
    The resulting [128, 128] sign tiles are the stationary operand in w's
    natural [K, units] layout (no transposes anywhere on-chip).
  - The matmul runs in bf16 on the PE (1 cycle/row vs 4 for fp32) and
    accumulates fp32 in PSUM over the full K=4096, so the only precision loss
    is the bf16 rounding of x (~2e-3 rel).
  - With units on the PSUM partition dim, the bias is per-partition: eviction
    is a single fused DVE op, out = max(psum + b[:,None], 0), with b exact in
    fp32. No bias matmuls, no extra relu pass.
  - The host transposes each core's [4096, 1024] outT back when assembling the
    full [8192, 4096] output.
"""

import numpy as np

import concourse.bass as bass
import concourse.tile as tile
from concourse import bacc
import concourse.mybir as mybir

f32 = mybir.dt.float32
bf16 = mybir.dt.bfloat16

B, D_IN, UNITS = 8192, 4096, 4096
N_CORES = 8
B_SH = B // N_CORES  # batch rows per core
P = 128


def build(b_sh=B_SH, d_in=D_IN, units=UNITS, n_chunk=512, m_tile=512, psum_bufs=1,
          repeats=1):
    ko_n = d_in // P        # contraction tiles of 128
    no_n = units // n_chunk  # unit chunks (sign-production granularity)
    nb_n = n_chunk // P     # 128-wide unit blocks per chunk (PSUM partition dim)
    mb_n = b_sh // m_tile   # batch blocks (PSUM free dim)
    assert ko_n >= 1 and no_n >= 1 and nb_n >= 1 and mb_n >= 1

    nc = bacc.Bacc(
        "TRN2", target_bir_lowering=False, debug=False, enable_asserts=False
    )
    xT = nc.dram_tensor("xT", [d_in, b_sh], f32, kind="ExternalInput").ap()
    w = nc.dram_tensor("w", [d_in, units], f32, kind="ExternalInput").ap()
    b = nc.dram_tensor("b", [1, units], f32, kind="ExternalInput").ap()
    outT = nc.dram_tensor("outT", [units, b_sh], f32, kind="ExternalOutput").ap()

    with tile.TileContext(nc) as tc:
        with (
            tc.tile_pool(name="xpool", bufs=1) as xpool,
            tc.tile_pool(name="xstage", bufs=3) as xstage,
            tc.tile_pool(name="spool", bufs=8) as spool,
            tc.tile_pool(name="wstage", bufs=8) as wstage,
            tc.tile_pool(name="biasp", bufs=1) as biasp,
            tc.tile_pool(name="opool", bufs=4) as opool,
            tc.tile_pool(name="psum", bufs=4, space="PSUM") as psum_pool,
        ):
            def body():
                # bias, laid out per-partition: b_sb[p, j] = b[j*128 + p]
                b_sb = biasp.tile([P, units // P], f32)
                nc.sync.dma_start(
                    out=b_sb, in_=b.rearrange("1 (j p) -> p j", p=P)
                )

                # x shard: cast to bf16, kept resident all kernel. The loads
                # are interleaved into the first unit-chunk's k-loop below so
                # the DMA queue alternates xT / w chunks and the PE can start
                # immediately instead of sitting behind the full x load.
                xT_sb = xpool.tile([P, ko_n, b_sh], bf16)
                xTr = xT.rearrange("(ko p) m -> ko p m", p=P)

                wr = w.rearrange("(ko p) n -> ko p n", p=P)
                # k-outer ordering: all nb*mb PSUM banks of one unit-chunk
                # accumulate concurrently, so the PE has a full chunk of work
                # per arriving k-tile and sign tiles are consumed just-in-time.
                for no in range(no_n):
                    pss = [
                        psum_pool.tile(
                            [P, m_tile], f32, name=f"ps_{g}", tag=f"ps_{g}",
                            bufs=psum_bufs,
                        )
                        for g in range(nb_n * mb_n)
                    ]
                    for ko in range(ko_n):
                        if no == 0:
                            xs = xstage.tile([P, b_sh], f32)
                            nc.sync.dma_start(out=xs, in_=xTr[ko])
                            nc.vector.tensor_copy(xT_sb[:, ko, :], xs)
                        ws = wstage.tile([P, n_chunk], f32)
                        nc.sync.dma_start(
                            out=ws,
                            in_=wr[ko, :, no * n_chunk : (no + 1) * n_chunk],
                        )
                        # binarize: fp32 -> sign -> bf16 (exact)
                        s_sb = spool.tile([P, n_chunk], bf16)
                        nc.scalar.sign(s_sb, ws)
                        # mb outer / nb inner: consecutive matmuls use
                        # different stationary tiles (measured ~6% faster MM
                        # cadence than back-to-back same-weight pairs)
                        for mb in range(mb_n):
                            for nb in range(nb_n):
                                nc.tensor.matmul(
                                    pss[nb * mb_n + mb],
                                    s_sb[:, nb * P : (nb + 1) * P],
                                    xT_sb[:, ko, mb * m_tile : (mb + 1) * m_tile],
                                    start=(ko == 0),
                                    stop=(ko == ko_n - 1),
                                )
                    for nb in range(nb_n):
                        n0 = no * n_chunk + nb * P  # global unit offset
                        for mb in range(mb_n):
                            g = nb * mb_n + mb
                            ot = opool.tile([P, m_tile], f32)
                            b_col = b_sb[:, n0 // P : n0 // P + 1]
                            # fused bias + relu: max(psum + b, 0). Alternate
                            # engines so banks free twice as fast at chunk
                            # boundaries (Sign and Relu share an ACT table
                            # set, so no table reloads).
                            if g % 2 == 0:
                                nc.vector.tensor_scalar(
                                    ot,
                                    pss[g],
                                    b_col,
                                    0.0,
                                    op0=mybir.AluOpType.add,
                                    op1=mybir.AluOpType.max,
                                )
                            else:
                                nc.scalar.activation(
                                    ot,
                                    pss[g],
                                    mybir.ActivationFunctionType.Relu,
                                    bias=b_col,
                                )
                            nc.sync.dma_start(
                                out=outT[
                                    n0 : n0 + P,
                                    mb * m_tile : (mb + 1) * m_tile,
                                ],
                                in_=ot,
                            )

            if repeats == 1:
                body()
            else:
                with tc.For_i(0, repeats, 1):
                    body()

    nc.compile()
    return nc


_nc_full = None


def _get_nc():
    global _nc_full
    if _nc_full is None:
        _nc_full = build()
    return _nc_full


def kernel(x, w, b):
    from concourse.bass_utils import run_bass_kernel_spmd

    x = np.ascontiguousarray(np.asarray(x, dtype=np.float32))
    w = np.ascontiguousarray(np.asarray(w, dtype=np.float32))
    b = np.ascontiguousarray(np.asarray(b, dtype=np.float32))
    assert x.shape == (B, D_IN) and w.shape == (D_IN, UNITS) and b.shape == (UNITS,)

    nc = _get_nc()
    b2 = b.reshape(1, UNITS)
    in_maps = []
    for c in range(N_CORES):
        xT = np.ascontiguousarray(x[c * B_SH : (c + 1) * B_SH].T)
        in_maps.append({"xT": xT, "w": w, "b": b2})
    res = run_bass_kernel_spmd(nc, in_maps, core_ids=list(range(N_CORES)))
    return np.concatenate(
        [np.ascontiguousarray(r["outT"].T) for r in res.results], axis=0
    )


# revision 12
# speedup vs baseline: 1.0275x; 1.0275x over previous
"""Bipolar dense layer on 8 Trainium2 NeuronCores.

Computes out = relu(x @ sign(w) + b) for x:[8192,4096] f32, w:[4096,4096] f32,
b:[4096] f32.

Strategy: data-parallel over the batch dim — each of the 8 cores gets a
[1024, 4096] shard of x (host passes it pre-transposed to [4096, 1024] so the
contraction dim lands on SBUF partitions), plus a full copy of w and b.

Per core (computing the TRANSPOSED output outT = [units, batch_shard]):
  - x shard is loaded once, cast fp32->bf16, and kept resident in SBUF (8 MB);
    its [128, 512] k-tiles are the matmul's moving operand.
  - w is streamed in [128, 512] fp32 tiles; sign() runs on the scalar engine
    (ACT) with a bf16 output — sign values {-1, 0, +1} are exact in bf16.
    The resulting [128, 128] sign tiles are the stationary operand in w's
    natural [K, units] layout (no transposes anywhere on-chip).
  - The matmul runs in bf16 on the PE (1 cycle/row vs 4 for fp32) and
    accumulates fp32 in PSUM over the full K=4096, so the only precision loss
    is the bf16 rounding of x (~2e-3 rel).
  - With units on the PSUM partition dim, the bias is per-partition: eviction
    is a single fused DVE op, out = max(psum + b[:,None], 0), with b exact in
    fp32. No bias matmuls, no extra relu pass.
  - The host transposes each core's [4096, 1024] outT back when assembling the
    full [8192, 4096] output.
"""

import numpy as np

import concourse.bass as bass
import concourse.tile as tile
from concourse import bacc
import concourse.mybir as mybir

f32 = mybir.dt.float32
bf16 = mybir.dt.bfloat16

B, D_IN, UNITS = 8192, 4096, 4096
N_CORES = 8
B_SH = B // N_CORES  # batch rows per core
P = 128


def build(b_sh=B_SH, d_in=D_IN, units=UNITS, n_chunk=512, m_tile=512, psum_bufs=1,
          repeats=1):
    ko_n = d_in // P        # contraction tiles of 128
    no_n = units // n_chunk  # unit chunks (sign-production granularity)
    nb_n = n_chunk // P     # 128-wide unit blocks per chunk (PSUM partition dim)
    mb_n = b_sh // m_tile   # batch blocks (PSUM free dim)
    assert ko_n >= 1 and no_n >= 1 and nb_n >= 1 and mb_n >= 1

    nc = bacc.Bacc(
        "TRN2", target_bir_lowering=False, debug=False, enable_asserts=False
    )
    xT = nc.dram_tensor("xT", [d_in, b_sh], f32, kind="ExternalInput").ap()
    w = nc.dram_tensor("w", [d_in, units], f32, kind="ExternalInput").ap()
    b = nc.dram_tensor("b", [1, units], f32, kind="ExternalInput").ap()
    outT = nc.dram_tensor("outT", [units, b_sh], f32, kind="ExternalOutput").ap()

    with tile.TileContext(nc) as tc:
        with (
            tc.tile_pool(name="xpool", bufs=1) as xpool,
            tc.tile_pool(name="xstage", bufs=3) as xstage,
            tc.tile_pool(name="spool", bufs=8) as spool,
            tc.tile_pool(name="wstage", bufs=8) as wstage,
            tc.tile_pool(name="biasp", bufs=1) as biasp,
            tc.tile_pool(name="opool", bufs=4) as opool,
            tc.tile_pool(name="psum", bufs=4, space="PSUM") as psum_pool,
        ):
            def body():
                # bias, laid out per-partition: b_sb[p, j] = b[j*128 + p]
                b_sb = biasp.tile([P, units // P], f32)
                nc.sync.dma_start(
                    out=b_sb, in_=b.rearrange("1 (j p) -> p j", p=P)
                )

                # x shard: cast to bf16, kept resident all kernel. The loads
                # are interleaved into the first unit-chunk's k-loop below so
                # the DMA queue alternates xT / w chunks and the PE can start
                # immediately instead of sitting behind the full x load.
                xT_sb = xpool.tile([P, ko_n, b_sh], bf16)
                xTr = xT.rearrange("(ko p) m -> ko p m", p=P)

                wr = w.rearrange("(ko p) n -> ko p n", p=P)
                # k-outer ordering: all nb*mb PSUM banks of one unit-chunk
                # accumulate concurrently, so the PE has a full chunk of work
                # per arriving k-tile and sign tiles are consumed just-in-time.
                for no in range(no_n):
                    pss = [
                        psum_pool.tile(
                            [P, m_tile], f32, name=f"ps_{g}", tag=f"ps_{g}",
                            bufs=psum_bufs,
                        )
                        for g in range(nb_n * mb_n)
                    ]
                    for ko in range(ko_n):
                        if no == 0:
                            xs = xstage.tile([P, b_sh], f32)
                            nc.sync.dma_start(out=xs, in_=xTr[ko])
                            nc.vector.tensor_copy(xT_sb[:, ko, :], xs)
                        ws = wstage.tile([P, n_chunk], f32)
                        nc.sync.dma_start(
                            out=ws,
                            in_=wr[ko, :, no * n_chunk : (no + 1) * n_chunk],
                        )
                        # binarize: fp32 -> sign -> bf16 (exact)
                        s_sb = spool.tile([P, n_chunk], bf16)
                        nc.scalar.sign(s_sb, ws)
                        for nb in range(nb_n):
                            for mb in range(mb_n):
                                nc.tensor.matmul(
                                    pss[nb * mb_n + mb],
                                    s_sb[:, nb * P : (nb + 1) * P],
                                    xT_sb[:, ko, mb * m_tile : (mb + 1) * m_tile],
                                    start=(ko == 0),
                                    stop=(ko == ko_n - 1),
                                )
                    for nb in range(nb_n):
                        n0 = no * n_chunk + nb * P  # global unit offset
                        for mb in range(mb_n):
                            g = nb * mb_n + mb
                            ot = opool.tile([P, m_tile], f32)
                            b_col = b_sb[:, n0 // P : n0 // P + 1]
                            # fused bias + relu: max(psum + b, 0). Alternate
                            # engines so banks free twice as fast at chunk
                            # boundaries (Sign and Relu share an ACT table
                            # set, so no table reloads).
                            if g % 2 == 0:
                                nc.vector.tensor_scalar(
                                    ot,
                                    pss[g],
                                    b_col,
                                    0.0,
                                    op0=mybir.AluOpType.add,
                                    op1=mybir.AluOpType.max,
                                )
                            else:
                                nc.scalar.activation(
                                    ot,
                                    pss[g],
                                    mybir.ActivationFunctionType.Relu,
                                    bias=b_col,
                                )
                            nc.sync.dma_start(
                                out=outT[
                                    n0 : n0 + P,
                                    mb * m_tile : (mb + 1) * m_tile,
                                ],
                                in_=ot,
                            )

            if repeats == 1:
                body()
            else:
                with tc.For_i(0, repeats, 1):
                    body()

    nc.compile()
    return nc


_nc_full = None


def _get_nc():
    global _nc_full
    if _nc_full is None:
        _nc_full = build()
    return _nc_full


def kernel(x, w, b):
    from concourse.bass_utils import run_bass_kernel_spmd

    x = np.ascontiguousarray(np.asarray(x, dtype=np.float32))
    w = np.ascontiguousarray(np.asarray(w, dtype=np.float32))
    b = np.ascontiguousarray(np.asarray(b, dtype=np.float32))
    assert x.shape == (B, D_IN) and w.shape == (D_IN, UNITS) and b.shape == (UNITS,)

    nc = _get_nc()
    b2 = b.reshape(1, UNITS)
    in_maps = []
    for c in range(N_CORES):
        xT = np.ascontiguousarray(x[c * B_SH : (c + 1) * B_SH].T)
        in_maps.append({"xT": xT, "w": w, "b": b2})
    res = run_bass_kernel_spmd(nc, in_maps, core_ids=list(range(N_CORES)))
    return np.concatenate(
        [np.ascontiguousarray(r["outT"].T) for r in res.results], axis=0
    )
